# revision 2
# baseline (speedup 1.0000x reference)
"""Trainium2 Bass kernel for nn_MultiHead (conv-conv-fc-mlp + per-sample head routing).

Network (per sample):
  conv1: [3,84,84] --3x3/s2--> [32,41,41] relu
  conv2: [32,41,41] --2x2/s1--> [64,40,40] relu
  fc:    [102400] -> [32] relu
  cat y(4) -> cb1 [36]->[32] relu -> cb2 [32]->[12] relu
  head routing: out[a] = r . heads_w[min(head,7), a, :] + heads_b[min(head,7), a]

Data-parallel over 8 NeuronCores (256 samples each). All heavy matmuls in bf16
(fp32 PSUM accumulation); small tail layers in fp32.

Key layout trick: conv1 is computed as ONE K=63 matmul whose output partition
layout [(dj',co), (i1,j2)] is exactly the rhs layout conv2 needs, so no
partition reshuffle is ever done on-chip.  conv2 output layout [(l,co),(i,j2)]
is likewise directly the fc chunk layout.
"""

import sys
import numpy as np

try:
    import concourse  # noqa: F401
except ImportError:
    sys.path.insert(0, "/opt/trn_rl_repo")

import ml_dtypes

import concourse.bass as bass  # noqa: F401
import concourse.mybir as mybir
import concourse.tile as tile
from concourse import bacc
from concourse.bass_utils import run_bass_kernel_spmd

BF16 = ml_dtypes.bfloat16
N_CORES = 8
B = 2048


# ----------------------------------------------------------------------------
# Device kernel builder
# ----------------------------------------------------------------------------

def build_nc(PB=256, NB=64, CH=8):
    """Build the per-core bass module. PB samples/core, NB samples/fc-group,
    CH samples per T0 DMA chunk."""
    assert PB % NB == 0 and NB % CH == 0
    G = PB // NB
    dt = mybir.dt
    AF = mybir.ActivationFunctionType
    ALU = mybir.AluOpType

    nc = bacc.Bacc("TRN2", target_bir_lowering=False, debug=False)

    t0_d = nc.dram_tensor("t0", [63, PB * 820], dt.bfloat16, kind="ExternalInput")
    yt_d = nc.dram_tensor("yt", [4, PB], dt.float32, kind="ExternalInput")
    m4_d = nc.dram_tensor("m4", [32, PB], dt.float32, kind="ExternalInput")
    w1_d = nc.dram_tensor("w1", [63, 96], dt.bfloat16, kind="ExternalInput")
    b1_d = nc.dram_tensor("b1", [96, 1], dt.float32, kind="ExternalInput")
    w2_d = nc.dram_tensor("w2", [96, 256], dt.bfloat16, kind="ExternalInput")
    b2_d = nc.dram_tensor("b2", [128, 1], dt.float32, kind="ExternalInput")
    fcw_d = nc.dram_tensor("fcw", [128, 25600], dt.bfloat16, kind="ExternalInput")
    fcb_d = nc.dram_tensor("fcb", [32, 1], dt.float32, kind="ExternalInput")
    c1t_d = nc.dram_tensor("c1t", [36, 32], dt.float32, kind="ExternalInput")
    c1b_d = nc.dram_tensor("c1b", [32, 1], dt.float32, kind="ExternalInput")
    c2t_d = nc.dram_tensor("c2t", [32, 12], dt.float32, kind="ExternalInput")
    c2b_d = nc.dram_tensor("c2b", [12, 1], dt.float32, kind="ExternalInput")
    hwt_d = nc.dram_tensor("hwt", [12, 32], dt.float32, kind="ExternalInput")
    hb_d = nc.dram_tensor("hb", [32, 1], dt.float32, kind="ExternalInput")
    r_d = nc.dram_tensor("r", [32, 4], dt.float32, kind="ExternalInput")
    out_d = nc.dram_tensor("outT", [4, PB], dt.float32, kind="ExternalOutput")

    with tile.TileContext(nc) as tc:
        with (
            tc.tile_pool(name="wpool", bufs=1) as wp,
            tc.tile_pool(name="iopool", bufs=2) as iop,
            tc.tile_pool(name="r1pool", bufs=3) as r1p,
            tc.tile_pool(name="s2pool", bufs=1) as s2p,
            tc.tile_pool(name="smpool", bufs=2) as smp,
            tc.tile_pool(name="pp1", bufs=3, space="PSUM") as pp1,
            tc.tile_pool(name="pp2", bufs=3, space="PSUM") as pp2,
            tc.tile_pool(name="ppf", bufs=1, space="PSUM") as ppf,
            tc.tile_pool(name="pps", bufs=1, space="PSUM") as pps,
        ):
            # ---- resident weights / constants ----
            w1s = wp.tile([63, 96], dt.bfloat16)
            nc.sync.dma_start(out=w1s[:], in_=w1_d[:])
            b1s = wp.tile([96, 1], dt.float32)
            nc.sync.dma_start(out=b1s[:], in_=b1_d[:])
            w2s = wp.tile([96, 256], dt.bfloat16)
            nc.sync.dma_start(out=w2s[:], in_=w2_d[:])
            b2s = wp.tile([128, 1], dt.float32)
            nc.sync.dma_start(out=b2s[:], in_=b2_d[:])
            fcws = wp.tile([128, 25600], dt.bfloat16)
            nc.sync.dma_start(out=fcws[:], in_=fcw_d[:])
            fcbs = wp.tile([32, 1], dt.float32)
            nc.sync.dma_start(out=fcbs[:], in_=fcb_d[:])
            c1ts = wp.tile([36, 32], dt.float32)
            nc.sync.dma_start(out=c1ts[:], in_=c1t_d[:])
            c1bs = wp.tile([32, 1], dt.float32)
            nc.sync.dma_start(out=c1bs[:], in_=c1b_d[:])
            c2ts = wp.tile([32, 12], dt.float32)
            nc.sync.dma_start(out=c2ts[:], in_=c2t_d[:])
            c2bs = wp.tile([12, 1], dt.float32)
            nc.sync.dma_start(out=c2bs[:], in_=c2b_d[:])
            hwts = wp.tile([12, 32], dt.float32)
            nc.sync.dma_start(out=hwts[:], in_=hwt_d[:])
            hbs = wp.tile([32, 1], dt.float32)
            nc.sync.dma_start(out=hbs[:], in_=hb_d[:])
            rs = wp.tile([32, 4], dt.float32)
            nc.sync.dma_start(out=rs[:], in_=r_d[:])
            m4s = wp.tile([32, PB], dt.float32)
            nc.sync.dma_start(out=m4s[:], in_=m4_d[:])
            # fc-input+y concat buffer; y goes to partitions 32:36 once.
            hfc = wp.tile([36, PB], dt.float32)
            nc.sync.dma_start(out=hfc[32:36, :], in_=yt_d[:])

            # S2: conv2 outputs for one group, [(l,co), (b_local, t)] bf16
            s2t = s2p.tile([128, NB * 800], dt.bfloat16)
            s2r = s2t.rearrange("p (b t) -> p b t", t=800)

            for g in range(G):
                b0 = g * NB
                # -------- conv phase --------
                for c0 in range(0, NB, CH):
                    t0t = iop.tile([63, CH * 820], dt.bfloat16, tag="t0t")
                    nc.sync.dma_start(
                        out=t0t[:],
                        in_=t0_d[:, (b0 + c0) * 820 : (b0 + c0 + CH) * 820],
                    )
                    for s in range(CH):
                        sl = s * 820
                        # conv1: K=63 -> psum [(dj',co), (i1,j2)] in 2 bank-chunks
                        ps1a = pp1.tile([96, 410], dt.float32, tag="ps1")
                        ps1b = pp1.tile([96, 410], dt.float32, tag="ps1")
                        nc.tensor.matmul(
                            ps1a[:], w1s[:], t0t[:, sl : sl + 410],
                            start=True, stop=True,
                        )
                        nc.tensor.matmul(
                            ps1b[:], w1s[:], t0t[:, sl + 410 : sl + 820],
                            start=True, stop=True,
                        )
                        # relu+bias eviction -> rhs1 (bf16), split ACT / DVE
                        r1t = r1p.tile([96, 820], dt.bfloat16, tag="r1t")
                        nc.scalar.activation(
                            r1t[:, 0:410], ps1a[:], AF.Relu, bias=b1s[:, 0:1],
                        )
                        nc.vector.tensor_scalar(
                            r1t[:, 410:820], ps1b[:],
                            scalar1=b1s[:, 0:1], scalar2=0.0,
                            op0=ALU.add, op1=ALU.max,
                        )
                        # conv2: 2 taps (di) accumulate; out [(l,co), (i,j2)]
                        ps2a = pp2.tile([128, 400], dt.float32, tag="ps2")
                        ps2b = pp2.tile([128, 400], dt.float32, tag="ps2")
                        nc.tensor.matmul(
                            ps2a[:], w2s[:, 0:128], r1t[:, 0:400],
                            start=True, stop=False, skip_group_check=True,
                        )
                        nc.tensor.matmul(
                            ps2b[:], w2s[:, 0:128], r1t[:, 400:800],
                            start=True, stop=False, skip_group_check=True,
                        )
                        nc.tensor.matmul(
                            ps2a[:], w2s[:, 128:256], r1t[:, 20:420],
                            start=False, stop=True, skip_group_check=True,
                        )
                        nc.tensor.matmul(
                            ps2b[:], w2s[:, 128:256], r1t[:, 420:820],
                            start=False, stop=True, skip_group_check=True,
                        )
                        col = (c0 + s) * 800
                        nc.scalar.activation(
                            s2t[:, col : col + 400], ps2a[:], AF.Relu,
                            bias=b2s[:, 0:1],
                        )
                        nc.vector.tensor_scalar(
                            s2t[:, col + 400 : col + 800], ps2b[:],
                            scalar1=b2s[:, 0:1], scalar2=0.0,
                            op0=ALU.add, op1=ALU.max,
                        )

                # -------- fc phase: 800 K=128 chunks accumulate [32, NB] ----
                psf = ppf.tile([32, NB], dt.float32, tag="psf")
                for t in range(800):
                    nc.tensor.matmul(
                        psf[:], fcws[:, t * 32 : (t + 1) * 32], s2r[:, :, t],
                        start=(t == 0), stop=(t == 799),
                    )
                nc.scalar.activation(
                    hfc[0:32, b0 : b0 + NB], psf[:], AF.Relu, bias=fcbs[:, 0:1],
                )

                # -------- tail: cb1 -> cb2 -> heads -> routed select --------
                psc1 = pps.tile([32, NB], dt.float32, tag="psmall")
                nc.tensor.matmul(
                    psc1[:], c1ts[:], hfc[:, b0 : b0 + NB], start=True, stop=True,
                )
                r1c = smp.tile([32, NB], dt.float32, tag="r1c")
                nc.vector.tensor_scalar(
                    r1c[:], psc1[:], scalar1=c1bs[:, 0:1], scalar2=0.0,
                    op0=ALU.add, op1=ALU.max,
                )
                psc2 = pps.tile([12, NB], dt.float32, tag="psmall")
                nc.tensor.matmul(psc2[:], c2ts[:], r1c[:], start=True, stop=True)
                r2c = smp.tile([12, NB], dt.float32, tag="r2c")
                nc.vector.tensor_scalar(
                    r2c[:], psc2[:], scalar1=c2bs[:, 0:1], scalar2=0.0,
                    op0=ALU.add, op1=ALU.max,
                )
                psah = pps.tile([32, NB], dt.float32, tag="psmall")
                nc.tensor.matmul(psah[:], hwts[:], r2c[:], start=True, stop=True)
                allh = smp.tile([32, NB], dt.float32, tag="allh")
                nc.vector.tensor_scalar(
                    allh[:], psah[:], scalar1=hbs[:, 0:1], scalar2=None,
                    op0=ALU.add,
                )
                masked = smp.tile([32, NB], dt.float32, tag="masked")
                nc.vector.tensor_tensor(
                    masked[:], allh[:], m4s[:, b0 : b0 + NB], op=ALU.mult,
                )
                pssel = pps.tile([4, NB], dt.float32, tag="psmall")
                nc.tensor.matmul(pssel[:], rs[:], masked[:], start=True, stop=True)
                ot = smp.tile([4, NB], dt.float32, tag="ot")
                nc.vector.tensor_copy(ot[:], pssel[:])
                nc.sync.dma_start(out=out_d[:, b0 : b0 + NB], in_=ot[:])

    nc.compile()
    return nc


# ----------------------------------------------------------------------------
# Host-side preprocessing
# ----------------------------------------------------------------------------

def prep_weights(conv1_w, conv1_b, conv2_w, conv2_b, fc_w, fc_b,
                 cb1_w, cb1_b, cb2_w, cb2_b, heads_w, heads_b):
    f32 = np.float32
    conv1_w = np.asarray(conv1_w, f32)
    conv2_w = np.asarray(conv2_w, f32)
    fc_w = np.asarray(fc_w, f32)

    # conv1 lhsT: [p=(e,di,ci), m=(dj',co)]
    L1 = np.zeros((7, 3, 3, 3, 32), f32)
    for djp in range(3):
        for dj in range(3):
            # L1[e, di, ci, djp, co] = w1[co, ci, di, dj],  e = 2*djp + dj
            L1[2 * djp + dj, :, :, djp, :] = conv1_w[:, :, :, dj].transpose(2, 1, 0)
    w1 = np.ascontiguousarray(L1.reshape(63, 96)).astype(BF16)
    b1 = np.tile(np.asarray(conv1_b, f32), 3).reshape(96, 1)

    # conv2 lhsT: [p=(dj',ci), cols=(di, l, co)]
    L2 = np.zeros((3, 32, 2, 2, 64), f32)
    for l in range(2):
        for dj in range(2):
            # L2[l+dj, ci, di, l, co] = w2[co, ci, di, dj]
            L2[l + dj, :, :, l, :] = conv2_w[:, :, :, dj].transpose(1, 2, 0)
    w2 = np.ascontiguousarray(L2.transpose(0, 1, 2, 3, 4).reshape(96, 256)).astype(BF16)
    # cols must be (di, l, co): L2 axes are (djp, ci, di, l, co) -> reshape OK
    b2 = np.tile(np.asarray(conv2_b, f32), 2).reshape(128, 1)

    # fc lhsT chunks: [k=(l,co), cols=(t=(i,j2), o)]
    fw5 = fc_w.reshape(32, 64, 40, 20, 2)
    fcw = np.ascontiguousarray(fw5.transpose(4, 1, 2, 3, 0).reshape(128, 25600)).astype(BF16)
    fcb = np.asarray(fc_b, f32).reshape(32, 1)

    c1t = np.ascontiguousarray(np.asarray(cb1_w, f32).T)          # [36,32]
    c1b = np.asarray(cb1_b, f32).reshape(32, 1)
    c2t = np.ascontiguousarray(np.asarray(cb2_w, f32).T)          # [32,12]
    c2b = np.asarray(cb2_b, f32).reshape(12, 1)
    hwt = np.ascontiguousarray(
        np.asarray(heads_w, f32).transpose(2, 0, 1).reshape(12, 32))
    hb = np.asarray(heads_b, f32).reshape(32, 1)
    r = np.ascontiguousarray(np.tile(np.eye(4, dtype=f32), (8, 1)))  # [32,4]
    return dict(w1=w1, b1=b1, w2=w2, b2=b2, fcw=fcw, fcb=fcb,
                c1t=c1t, c1b=c1b, c2t=c2t, c2b=c2b, hwt=hwt, hb=hb, r=r)


def prep_x_im2col(xc):
    """xc: [PB, 3, 84, 84] f32 -> T0 [63, PB*820] bf16 with
    T0[(e*9+di*3+ci), b*820 + i1*20 + j2] = x[b, ci, 2*i1+di, 4*j2+e]."""
    xc = np.ascontiguousarray(np.asarray(xc, np.float32))
    PB = xc.shape[0]
    sb, sc, sh, sw = xc.strides
    v = np.lib.stride_tricks.as_strided(
        xc, shape=(PB, 7, 3, 3, 41, 20),
        strides=(sb, sw, sh, sc, 2 * sh, 4 * sw))
    t0 = v.transpose(1, 2, 3, 0, 4, 5).reshape(63, PB * 820)
    return np.ascontiguousarray(t0).astype(BF16)


def prep_per_core(x, y, head, PB):
    """Split batch across cores and build per-core input dicts."""
    x = np.asarray(x, np.float32)
    y = np.asarray(y, np.float32)
    head = np.asarray(head)
    Btot = x.shape[0]
    assert Btot == N_CORES * PB
    idxc = np.minimum(head, 7).astype(np.int64)
    hp = (np.arange(32) // 4)
    mask4 = (idxc[None, :] == hp[:, None]).astype(np.float32)     # [32, B]
    yT = np.ascontiguousarray(y.T)                                 # [4, B]
    per_core = []
    for c in range(N_CORES):
        sl = slice(c * PB, (c + 1) * PB)
        per_core.append(dict(
            t0=prep_x_im2col(x[sl]),
            yt=np.ascontiguousarray(yT[:, sl]),
            m4=np.ascontiguousarray(mask4[:, sl]),
        ))
    return per_core


# ----------------------------------------------------------------------------
# Entry point
# ----------------------------------------------------------------------------

_NC_CACHE = {}


def get_nc(PB=256, NB=64, CH=8):
    key = (PB, NB, CH)
    if key not in _NC_CACHE:
        _NC_CACHE[key] = build_nc(PB, NB, CH)
    return _NC_CACHE[key]


def kernel(x, y, head, conv1_w, conv1_b, conv2_w, conv2_b, fc_w, fc_b,
           cb1_w, cb1_b, cb2_w, cb2_b, heads_w, heads_b):
    PB = B // N_CORES
    nc = get_nc(PB)
    w = prep_weights(conv1_w, conv1_b, conv2_w, conv2_b, fc_w, fc_b,
                     cb1_w, cb1_b, cb2_w, cb2_b, heads_w, heads_b)
    pc = prep_per_core(x, y, head, PB)
    in_maps = [{**w, **pc[c]} for c in range(N_CORES)]
    res = run_bass_kernel_spmd(nc, in_maps, core_ids=list(range(N_CORES)))
    out = np.empty((B, 4), np.float32)
    for c in range(N_CORES):
        out[c * PB : (c + 1) * PB, :] = res.results[c]["outT"].T
    return out
